# revision 24
# baseline (speedup 1.0000x reference)
"""Correlation cost volume kernel for Trainium2 (8 NeuronCores, batch-parallel).

cost[b, i, h, x] = mean_c left[b,c,h,x] * right[b,c,h,x-i], i in [0,48), zero for x < i.

Per core (one batch element):
  Inputs are host-cast to bf16, left pre-scaled by 1/128 (exact power of two),
  so no on-device scaling is needed and all DMA traffic is halved.
  For each h row and x-chunk (M=128/128/64): PSUM G[a, j] = sum_c
  lscaled[c, X0+a] * right[c, X0-47+j]. Right is loaded contiguously with
  slack; out-of-range columns read garbage that only reaches the x < i
  triangle, which the host masks to zero.
  PSUM tile [128, 1024] (2 banks; chunk slots at {0,256,512} so no matmul
  crosses a bank). Two DVE/ACT copies per h row cast to bf16 into the group
  rect tile, hl-major row blocks of 464 (A 176 | B 176 | C 112).
  Shear band[a, (hl*3+ci)*48 + k] = G[a, a+k]:
   - scatter groups: one gpsimd local_scatter per group (per-partition
     indices; invalid lanes zeroed) + one full-rate contiguous store.
     hl-major makes a 4-row group's index table a prefix of the full one,
     so the first/last groups are 4-row halves (earlier Pool start/finish).
   - dump groups: 10 quad-block DMAs store the 80-wide diagonal quarters;
     the host extracts the diagonals (no Pool time).
  Shear-store DMAs are emitted a few groups late on nc.sync so their waits
  are satisfied at issue time and never stall a sequencer.
  Host untangles layouts -> (i=47-k, h, x), flips i, zeroes x < i.
"""
import os

import numpy as np
import ml_dtypes

import concourse.bacc as bacc
import concourse.mybir as mybir
import concourse.tile as tile
from concourse.ap import AP
from concourse.bass_utils import run_bass_kernel_spmd

B, C, H, W = 8, 128, 96, 320
D = 48  # disparities
HW = H * W
CHUNKS = [(0, 128, 176), (128, 128, 176), (256, 64, 112)]  # (X0, M, NMM)
ROWW = 464  # scatter-group rect row block: A 176 | B 176 | C 112
DROWW = 528  # dump-group rect row block: 3 uniform 176 slots (C padded)
RECW = 8 * DROWW  # rect tile width (scatter groups use prefix 8*464)
RW = 47 + 8 * W + 48  # right tile width incl. slack (2655)
# (h0, nrows, kind): 'S' = gpsimd local_scatter shear, 'D' = quad dump (host shear)
GROUPS = [(0, 4, "S"), (4, 4, "S"), (8, 8, "S"), (16, 8, "S"), (24, 8, "S"),
          (32, 8, "D"), (40, 8, "S"), (48, 8, "S"), (56, 8, "S"), (64, 8, "D"),
          (72, 8, "S"), (80, 8, "S"), (88, 4, "S"), (92, 4, "S")]
SCATTERS = [(h0, nr) for h0, nr, kk in GROUPS if kk == "S"]
DUMPS = [(h0, nr) for h0, nr, kk in GROUPS if kk == "D"]
OUT2_OFF = np.cumsum([0] + [128 * nr * 144 for h0, nr in SCATTERS]).tolist()
QB = 32 * 24 * 80  # one quarter block: all 24 slots
QTOT = 4 * QB  # per dump group
STORE_LAG = 3

_cache = {}


def make_idxs():
    """idx[a, hl*464 + off_ci + col] = (hl*3+ci)*48 + (col-a) if valid else -1."""
    idx = np.full((128, 8 * ROWW), -1, dtype=np.int16)
    a = np.arange(128)
    for hl in range(8):
        for ci, off, cw in ((0, 0, 176), (1, 176, 176), (2, 352, 112)):
            s = hl * 3 + ci
            for k in range(D):
                col = a + k
                valid = col < cw
                if ci == 2:
                    valid = valid & (a < 64)
                idx[a[valid], hl * ROWW + off + col[valid]] = s * D + k
    return idx


def _emit_store(nc, out2, quads, item):
    kind, gi, tile_ = item
    if kind == "band":
        si = [j for j, (h0, nr, kk) in enumerate(GROUPS) if kk == "S"].index(gi)
        nrows = GROUPS[gi][1]
        dst = AP(out2.tensor, out2.offset + OUT2_OFF[si],
                 [[nrows * 144, 128], [1, nrows * 144]])
        nc.sync.dma_start(out=dst, in_=tile_[:, : nrows * 144])
        return
    rp = tile_.ap[0][0]
    di = [j for j, (h0, nr, kk) in enumerate(GROUPS) if kk == "D"].index(gi)
    qbase = quads.offset + di * QTOT
    for q in range(4):  # quarter q: rows [32q,32q+32), cols [32q,32q+80) of all 24 slots
        src = AP(tile_.tensor, tile_.offset + 32 * q * rp + 32 * q,
                 [[rp, 32], [176, 24], [1, 80]])
        dst = AP(quads.tensor, qbase + q * QB, [[24 * 80, 32], [80, 24], [1, 80]])
        nc.sync.dma_start(out=dst, in_=src)


def _build():
    nc = bacc.Bacc("TRN2", target_bir_lowering=False, debug=False, num_devices=8)
    left = nc.dram_tensor("left", [C, HW], mybir.dt.bfloat16, kind="ExternalInput").ap()
    right = nc.dram_tensor("right", [C, HW], mybir.dt.bfloat16, kind="ExternalInput").ap()
    idxs_in = nc.dram_tensor("idxs", [128, 8 * ROWW], mybir.dt.int16, kind="ExternalInput").ap()
    out2 = nc.dram_tensor("out2", [OUT2_OFF[-1]], mybir.dt.bfloat16,
                          kind="ExternalOutput").ap()
    quads = nc.dram_tensor("quads", [len(DUMPS) * QTOT], mybir.dt.bfloat16,
                           kind="ExternalOutput").ap()

    with tile.TileContext(nc) as tc:
        with (
            tc.tile_pool(name="io", bufs=8) as io_pool,
            tc.tile_pool(name="rect", bufs=8) as rect_pool,
            tc.tile_pool(name="band", bufs=8) as band_pool,
            tc.tile_pool(name="const", bufs=1) as const_pool,
            tc.tile_pool(name="ps", bufs=4, space="PSUM") as ps_pool,
        ):
            idx_t = const_pool.tile([128, 8 * ROWW], mybir.dt.int16)
            pending = []

            for gi, (h0, nrows, kind) in enumerate(GROUPS):
                l_t = io_pool.tile([C, 8 * W], mybir.dt.bfloat16, tag="lt")
                r_t = io_pool.tile([C, RW], mybir.dt.bfloat16, tag="rt")
                nc.sync.dma_start(out=l_t[:, : nrows * W],
                                  in_=left[:, h0 * W : (h0 + nrows) * W])
                nc.sync.dma_start(out=r_t[:, 47 : 47 + nrows * W],
                                  in_=right[:, h0 * W : (h0 + nrows) * W])
                # idx table loaded in prefix pieces so early small groups
                # can scatter before the whole table arrives
                if gi == 0:
                    nc.sync.dma_start(out=idx_t[:, : 2 * ROWW],
                                      in_=idxs_in[:, : 2 * ROWW])
                elif gi == 1:
                    nc.sync.dma_start(out=idx_t[:, 2 * ROWW : 4 * ROWW],
                                      in_=idxs_in[:, 2 * ROWW : 4 * ROWW])
                elif gi == 2:
                    nc.sync.dma_start(out=idx_t[:, 4 * ROWW :],
                                      in_=idxs_in[:, 4 * ROWW :])

                rect_g = rect_pool.tile([128, RECW], mybir.dt.bfloat16, tag="rect")
                rp = rect_g.ap[0][0]
                roww = ROWW if kind == "S" else DROWW
                for hl in range(nrows):
                    # 2 PSUM banks; chunk slots at {0,256,512}: no bank crossing.
                    g_ps = ps_pool.tile([128, 1024], mybir.dt.float32, tag="gps")
                    pp = g_ps.ap[0][0]
                    for ci, (X0, M, NMM) in enumerate(CHUNKS):
                        nc.tensor.matmul(
                            g_ps[:M, ci * 256 : ci * 256 + NMM],
                            l_t[:, hl * W + X0 : hl * W + X0 + M],
                            r_t[:, hl * W + X0 : hl * W + X0 + NMM],
                            start=True, stop=True,
                        )
                    dst_ab = AP(rect_g.tensor, rect_g.offset + hl * roww,
                                [[rp, 128], [176, 2], [1, 176]])
                    src_ab = AP(g_ps.tensor, g_ps.offset, [[pp, 128], [256, 2], [1, 176]])
                    dst_c = rect_g[:, hl * roww + 352 : hl * roww + 352 + 112]
                    src_c = g_ps[:, 512 : 512 + 112]
                    if hl % 2 == 0:
                        nc.vector.tensor_copy(dst_ab, src_ab)
                        nc.scalar.copy(dst_c, src_c)
                    else:
                        nc.scalar.copy(dst_ab, src_ab)
                        nc.vector.tensor_copy(dst_c, src_c)

                if kind == "S":
                    band_g = band_pool.tile([128, 8 * 144], mybir.dt.bfloat16, tag="band")
                    nc.gpsimd.local_scatter(
                        band_g[:, : nrows * 144], rect_g[:, : nrows * ROWW],
                        idx_t[:, : nrows * ROWW],
                        channels=128, num_elems=nrows * 144, num_idxs=nrows * ROWW,
                    )
                    pending.append(("band", gi, band_g))
                else:
                    pending.append(("dump", gi, rect_g))
                while pending and pending[0][1] <= gi - STORE_LAG:
                    _emit_store(nc, out2, quads, pending.pop(0))
            while pending:
                _emit_store(nc, out2, quads, pending.pop(0))
    nc.compile()
    return nc


def _get_nc(_mode=None):
    if "nc" not in _cache:
        _cache["nc"] = _build()
    return _cache["nc"]


def kernel(left_feature, right_feature):
    left_feature = np.asarray(left_feature, dtype=np.float32)
    right_feature = np.asarray(right_feature, dtype=np.float32)
    b, c, h, w = left_feature.shape
    assert (b, c, h, w) == (B, C, H, W)
    nc = _get_nc()
    idx = make_idxs()
    in_maps = []
    for i in range(B):
        lf = (left_feature[i].reshape(C, HW) * np.float32(1.0 / C)).astype(ml_dtypes.bfloat16)
        rf = right_feature[i].reshape(C, HW).astype(ml_dtypes.bfloat16)
        in_maps.append({
            "left": np.ascontiguousarray(lf),
            "right": np.ascontiguousarray(rf),
            "idxs": idx,
        })
    trace = bool(os.environ.get("KERNEL_TRACE"))
    res = run_bass_kernel_spmd(nc, in_maps, core_ids=list(range(B)), trace=trace)
    if trace:
        print("HW exec time:", res.exec_time_ns, "ns")
    outs = []
    a32 = np.arange(32)
    for i in range(B):
        vol = np.empty((D, H, W), dtype=np.float32)
        raw = np.asarray(res.results[i]["out2"]).astype(np.float32)
        for si, (h0, nr) in enumerate(SCATTERS):
            band = raw[OUT2_OFF[si] : OUT2_OFF[si + 1]].reshape(128, nr, 3, D)
            v = band.transpose(3, 1, 2, 0)  # [k, hl, ci, a]
            vol[:, h0 : h0 + nr, :] = v.reshape(D, nr, 3 * 128)[:, :, :W]
        qall = np.asarray(res.results[i]["quads"]).astype(np.float32).reshape(
            len(DUMPS), 4, 32, 24, 80)  # [dg, quarter, a32, slot=hl*3+ci, col80]
        for di, (h0, nr) in enumerate(DUMPS):
            for k in range(D):
                sel = (a32 + k)[None, :, None, None]
                blk = np.take_along_axis(qall[di], sel, axis=3)[:, :, :, 0]  # [4, 32, 24]
                bnd = blk.reshape(128, 8, 3)  # [a, hl, ci]
                vv = bnd.transpose(1, 2, 0).reshape(8, 3 * 128)[:, :W]  # [hl, x]
                vol[k, h0 : h0 + nr, :] = vv
        outs.append(vol[::-1])  # k = 47 - i
    out = np.stack(outs, axis=0)
    for i in range(1, D):
        out[:, i, :, :i] = 0.0
    return out


if __name__ == "__main__":
    rng = np.random.default_rng(0)
    lf = rng.standard_normal((B, C, H, W), dtype=np.float32)
    rf = rng.standard_normal((B, C, H, W), dtype=np.float32)
    got = kernel(lf, rf)
    for (bb, i, hh, xx) in [(0, 0, 0, 0), (0, 5, 10, 100), (1, 47, 95, 319), (2, 47, 3, 10),
                            (3, 20, 85, 200), (7, 1, 90, 300), (5, 30, 35, 128)]:
        want = float(np.dot(lf[bb, :, hh, xx], rf[bb, :, hh, xx - i]) / C) if xx >= i else 0.0
        print((bb, i, hh, xx), "got", got[bb, i, hh, xx], "want", want)


# revision 25
# speedup vs baseline: 1.0525x; 1.0525x over previous
"""Correlation cost volume kernel for Trainium2 (8 NeuronCores, batch-parallel).

cost[b, i, h, x] = mean_c left[b,c,h,x] * right[b,c,h,x-i], i in [0,48), zero for x < i.

Per core (one batch element):
  Inputs are host-cast to bf16, left pre-scaled by 1/128 (exact power of two),
  so no on-device scaling is needed and all DMA traffic is halved.
  For each h row and x-chunk (M=128/128/64): PSUM G[a, j] = sum_c
  lscaled[c, X0+a] * right[c, X0-47+j]. Right is loaded contiguously with
  slack; out-of-range columns read garbage that only reaches the x < i
  triangle, which the host masks to zero.
  PSUM tile [128, 1024] (2 banks; chunk slots at {0,256,512} so no matmul
  crosses a bank). Two DVE/ACT copies per h row cast to bf16 into the group
  rect tile, hl-major row blocks of 464 (A 176 | B 176 | C 112).
  Shear band[a, (hl*3+ci)*48 + k] = G[a, a+k]:
   - scatter groups: one gpsimd local_scatter per group (per-partition
     indices; invalid lanes zeroed) + one full-rate contiguous store.
     hl-major makes a 4-row group's index table a prefix of the full one,
     so the first/last groups are 4-row halves (earlier Pool start/finish).
   - dump groups: 10 quad-block DMAs store the 80-wide diagonal quarters;
     the host extracts the diagonals (no Pool time).
  Shear-store DMAs are emitted a few groups late on nc.sync so their waits
  are satisfied at issue time and never stall a sequencer.
  Host untangles layouts -> (i=47-k, h, x), flips i, zeroes x < i.
"""
import os

import numpy as np
import ml_dtypes

import concourse.bacc as bacc
import concourse.mybir as mybir
import concourse.tile as tile
from concourse.ap import AP
from concourse.bass_utils import run_bass_kernel_spmd

B, C, H, W = 8, 128, 96, 320
D = 48  # disparities
HW = H * W
CHUNKS = [(0, 128, 176), (128, 128, 176), (256, 64, 112)]  # (X0, M, NMM)
ROWW = 464  # scatter-group rect row block: A 176 | B 176 | C 112
DROWW = 528  # dump-group rect row block: 3 uniform 176 slots (C padded)
RECW = 8 * DROWW  # rect tile width (scatter groups use prefix 8*464)
RW = 47 + 8 * W + 48  # right tile width incl. slack (2655)
# (h0, nrows, kind): 'S' = gpsimd local_scatter shear, 'D' = quad dump (host shear)
GROUPS = [(0, 4, "S"), (4, 4, "S"), (8, 8, "S"), (16, 8, "S"), (24, 8, "S"),
          (32, 8, "D"), (40, 8, "S"), (48, 8, "S"), (56, 8, "S"), (64, 8, "D"),
          (72, 8, "S"), (80, 8, "S"), (88, 4, "S"), (92, 4, "S")]
SCATTERS = [(h0, nr) for h0, nr, kk in GROUPS if kk == "S"]
DUMPS = [(h0, nr) for h0, nr, kk in GROUPS if kk == "D"]
OUT2_OFF = np.cumsum([0] + [128 * nr * 144 for h0, nr in SCATTERS]).tolist()
QB = 32 * 24 * 80  # one quarter block: all 24 slots
QTOT = 4 * QB  # per dump group
STORE_LAG = 5

_cache = {}


def make_idxs():
    """idx[a, hl*464 + off_ci + col] = (hl*3+ci)*48 + (col-a) if valid else -1."""
    idx = np.full((128, 8 * ROWW), -1, dtype=np.int16)
    a = np.arange(128)
    for hl in range(8):
        for ci, off, cw in ((0, 0, 176), (1, 176, 176), (2, 352, 112)):
            s = hl * 3 + ci
            for k in range(D):
                col = a + k
                valid = col < cw
                if ci == 2:
                    valid = valid & (a < 64)
                idx[a[valid], hl * ROWW + off + col[valid]] = s * D + k
    return idx


def _emit_store(nc, out2, quads, item):
    kind, gi, tile_ = item
    if kind == "band":
        si = [j for j, (h0, nr, kk) in enumerate(GROUPS) if kk == "S"].index(gi)
        nrows = GROUPS[gi][1]
        dst = AP(out2.tensor, out2.offset + OUT2_OFF[si],
                 [[nrows * 144, 128], [1, nrows * 144]])
        nc.sync.dma_start(out=dst, in_=tile_[:, : nrows * 144])
        return
    rp = tile_.ap[0][0]
    di = [j for j, (h0, nr, kk) in enumerate(GROUPS) if kk == "D"].index(gi)
    qbase = quads.offset + di * QTOT
    for q in range(4):  # quarter q: rows [32q,32q+32), cols [32q,32q+80) of all 24 slots
        src = AP(tile_.tensor, tile_.offset + 32 * q * rp + 32 * q,
                 [[rp, 32], [176, 24], [1, 80]])
        dst = AP(quads.tensor, qbase + q * QB, [[24 * 80, 32], [80, 24], [1, 80]])
        nc.sync.dma_start(out=dst, in_=src)


def _build():
    nc = bacc.Bacc("TRN2", target_bir_lowering=False, debug=False, num_devices=8)
    left = nc.dram_tensor("left", [C, HW], mybir.dt.bfloat16, kind="ExternalInput").ap()
    right = nc.dram_tensor("right", [C, HW], mybir.dt.bfloat16, kind="ExternalInput").ap()
    idxs_in = nc.dram_tensor("idxs", [128, 8 * ROWW], mybir.dt.int16, kind="ExternalInput").ap()
    out2 = nc.dram_tensor("out2", [OUT2_OFF[-1]], mybir.dt.bfloat16,
                          kind="ExternalOutput").ap()
    quads = nc.dram_tensor("quads", [len(DUMPS) * QTOT], mybir.dt.bfloat16,
                           kind="ExternalOutput").ap()

    with tile.TileContext(nc) as tc:
        with (
            tc.tile_pool(name="io", bufs=8) as io_pool,
            tc.tile_pool(name="rect", bufs=8) as rect_pool,
            tc.tile_pool(name="band", bufs=8) as band_pool,
            tc.tile_pool(name="const", bufs=1) as const_pool,
            tc.tile_pool(name="ps", bufs=4, space="PSUM") as ps_pool,
        ):
            idx_t = const_pool.tile([128, 8 * ROWW], mybir.dt.int16)
            pending = []

            for gi, (h0, nrows, kind) in enumerate(GROUPS):
                l_t = io_pool.tile([C, 8 * W], mybir.dt.bfloat16, tag="lt")
                r_t = io_pool.tile([C, RW], mybir.dt.bfloat16, tag="rt")
                nc.sync.dma_start(out=l_t[:, : nrows * W],
                                  in_=left[:, h0 * W : (h0 + nrows) * W])
                nc.sync.dma_start(out=r_t[:, 47 : 47 + nrows * W],
                                  in_=right[:, h0 * W : (h0 + nrows) * W])
                # idx table loaded in prefix pieces so early small groups
                # can scatter before the whole table arrives
                if gi == 0:
                    nc.sync.dma_start(out=idx_t[:, : 2 * ROWW],
                                      in_=idxs_in[:, : 2 * ROWW])
                elif gi == 1:
                    nc.sync.dma_start(out=idx_t[:, 2 * ROWW : 4 * ROWW],
                                      in_=idxs_in[:, 2 * ROWW : 4 * ROWW])
                elif gi == 2:
                    nc.sync.dma_start(out=idx_t[:, 4 * ROWW :],
                                      in_=idxs_in[:, 4 * ROWW :])

                rect_g = rect_pool.tile([128, RECW], mybir.dt.bfloat16, tag="rect")
                rp = rect_g.ap[0][0]
                roww = ROWW if kind == "S" else DROWW
                for hl in range(nrows):
                    # 2 PSUM banks; chunk slots at {0,256,512}: no bank crossing.
                    g_ps = ps_pool.tile([128, 1024], mybir.dt.float32, tag="gps")
                    pp = g_ps.ap[0][0]
                    for ci, (X0, M, NMM) in enumerate(CHUNKS):
                        nc.tensor.matmul(
                            g_ps[:M, ci * 256 : ci * 256 + NMM],
                            l_t[:, hl * W + X0 : hl * W + X0 + M],
                            r_t[:, hl * W + X0 : hl * W + X0 + NMM],
                            start=True, stop=True,
                        )
                    dst_ab = AP(rect_g.tensor, rect_g.offset + hl * roww,
                                [[rp, 128], [176, 2], [1, 176]])
                    src_ab = AP(g_ps.tensor, g_ps.offset, [[pp, 128], [256, 2], [1, 176]])
                    dst_c = rect_g[:, hl * roww + 352 : hl * roww + 352 + 112]
                    src_c = g_ps[:, 512 : 512 + 112]
                    if hl % 2 == 0:
                        nc.vector.tensor_copy(dst_ab, src_ab)
                        nc.scalar.copy(dst_c, src_c)
                    else:
                        nc.scalar.copy(dst_ab, src_ab)
                        nc.vector.tensor_copy(dst_c, src_c)

                if kind == "S":
                    band_g = band_pool.tile([128, 8 * 144], mybir.dt.bfloat16, tag="band")
                    nc.gpsimd.local_scatter(
                        band_g[:, : nrows * 144], rect_g[:, : nrows * ROWW],
                        idx_t[:, : nrows * ROWW],
                        channels=128, num_elems=nrows * 144, num_idxs=nrows * ROWW,
                    )
                    pending.append(("band", gi, band_g))
                else:
                    pending.append(("dump", gi, rect_g))
                while pending and pending[0][1] <= gi - STORE_LAG:
                    _emit_store(nc, out2, quads, pending.pop(0))
            while pending:
                _emit_store(nc, out2, quads, pending.pop(0))
    nc.compile()
    return nc


def _get_nc(_mode=None):
    if "nc" not in _cache:
        _cache["nc"] = _build()
    return _cache["nc"]


def kernel(left_feature, right_feature):
    left_feature = np.asarray(left_feature, dtype=np.float32)
    right_feature = np.asarray(right_feature, dtype=np.float32)
    b, c, h, w = left_feature.shape
    assert (b, c, h, w) == (B, C, H, W)
    nc = _get_nc()
    idx = make_idxs()
    in_maps = []
    for i in range(B):
        lf = (left_feature[i].reshape(C, HW) * np.float32(1.0 / C)).astype(ml_dtypes.bfloat16)
        rf = right_feature[i].reshape(C, HW).astype(ml_dtypes.bfloat16)
        in_maps.append({
            "left": np.ascontiguousarray(lf),
            "right": np.ascontiguousarray(rf),
            "idxs": idx,
        })
    trace = bool(os.environ.get("KERNEL_TRACE"))
    res = run_bass_kernel_spmd(nc, in_maps, core_ids=list(range(B)), trace=trace)
    if trace:
        print("HW exec time:", res.exec_time_ns, "ns")
    outs = []
    a32 = np.arange(32)
    for i in range(B):
        vol = np.empty((D, H, W), dtype=np.float32)
        raw = np.asarray(res.results[i]["out2"]).astype(np.float32)
        for si, (h0, nr) in enumerate(SCATTERS):
            band = raw[OUT2_OFF[si] : OUT2_OFF[si + 1]].reshape(128, nr, 3, D)
            v = band.transpose(3, 1, 2, 0)  # [k, hl, ci, a]
            vol[:, h0 : h0 + nr, :] = v.reshape(D, nr, 3 * 128)[:, :, :W]
        qall = np.asarray(res.results[i]["quads"]).astype(np.float32).reshape(
            len(DUMPS), 4, 32, 24, 80)  # [dg, quarter, a32, slot=hl*3+ci, col80]
        for di, (h0, nr) in enumerate(DUMPS):
            for k in range(D):
                sel = (a32 + k)[None, :, None, None]
                blk = np.take_along_axis(qall[di], sel, axis=3)[:, :, :, 0]  # [4, 32, 24]
                bnd = blk.reshape(128, 8, 3)  # [a, hl, ci]
                vv = bnd.transpose(1, 2, 0).reshape(8, 3 * 128)[:, :W]  # [hl, x]
                vol[k, h0 : h0 + nr, :] = vv
        outs.append(vol[::-1])  # k = 47 - i
    out = np.stack(outs, axis=0)
    for i in range(1, D):
        out[:, i, :, :i] = 0.0
    return out


if __name__ == "__main__":
    rng = np.random.default_rng(0)
    lf = rng.standard_normal((B, C, H, W), dtype=np.float32)
    rf = rng.standard_normal((B, C, H, W), dtype=np.float32)
    got = kernel(lf, rf)
    for (bb, i, hh, xx) in [(0, 0, 0, 0), (0, 5, 10, 100), (1, 47, 95, 319), (2, 47, 3, 10),
                            (3, 20, 85, 200), (7, 1, 90, 300), (5, 30, 35, 128)]:
        want = float(np.dot(lf[bb, :, hh, xx], rf[bb, :, hh, xx - i]) / C) if xx >= i else 0.0
        print((bb, i, hh, xx), "got", got[bb, i, hh, xx], "want", want)


# revision 26
# speedup vs baseline: 1.0796x; 1.0258x over previous
"""Correlation cost volume kernel for Trainium2 (8 NeuronCores, batch-parallel).

cost[b, i, h, x] = mean_c left[b,c,h,x] * right[b,c,h,x-i], i in [0,48), zero for x < i.

Per core (one batch element):
  Inputs are host-cast to bf16, left pre-scaled by 1/128 (exact power of two),
  so no on-device scaling is needed and all DMA traffic is halved.
  For each h row and x-chunk (M=128/128/64): PSUM G[a, j] = sum_c
  lscaled[c, X0+a] * right[c, X0-47+j]. Right is loaded contiguously with
  slack; out-of-range columns read garbage that only reaches the x < i
  triangle, which the host masks to zero.
  PSUM tile [128, 1024] (2 banks; chunk slots at {0,256,512} so no matmul
  crosses a bank). Two DVE/ACT copies per h row cast to bf16 into the group
  rect tile, hl-major row blocks of 464 (A 176 | B 176 | C 112).
  Shear band[a, (hl*3+ci)*48 + k] = G[a, a+k]:
   - scatter groups: one gpsimd local_scatter per group (per-partition
     indices; invalid lanes zeroed) + one full-rate contiguous store.
     hl-major makes a 4-row group's index table a prefix of the full one,
     so the first/last groups are 4-row halves (earlier Pool start/finish).
   - dump groups: 10 quad-block DMAs store the 80-wide diagonal quarters;
     the host extracts the diagonals (no Pool time).
  Shear-store DMAs are emitted a few groups late on nc.sync so their waits
  are satisfied at issue time and never stall a sequencer.
  Host untangles layouts -> (i=47-k, h, x), flips i, zeroes x < i.
"""
import os

import numpy as np
import ml_dtypes

import concourse.bacc as bacc
import concourse.mybir as mybir
import concourse.tile as tile
from concourse.ap import AP
from concourse.bass_utils import run_bass_kernel_spmd

B, C, H, W = 8, 128, 96, 320
D = 48  # disparities
HW = H * W
CHUNKS = [(0, 128, 176), (128, 128, 176), (256, 64, 112)]  # (X0, M, NMM)
ROWW = 464  # scatter-group rect row block: A 176 | B 176 | C 112
DROWW = 528  # dump-group rect row block: 3 uniform 176 slots (C padded)
RECW = 8 * DROWW  # rect tile width (scatter groups use prefix 8*464)
RW = 47 + 8 * W + 48  # right tile width incl. slack (2655)
# (h0, nrows, kind): 'S' = gpsimd local_scatter shear, 'D' = quad dump (host shear)
GROUPS = [(0, 4, "S"), (4, 4, "S"), (8, 8, "S"), (16, 8, "S"), (24, 8, "S"),
          (32, 8, "D"), (40, 8, "S"), (48, 8, "S"), (56, 8, "S"), (64, 8, "D"),
          (72, 8, "S"), (80, 8, "S"), (88, 4, "S"), (92, 4, "S")]
SCATTERS = [(h0, nr) for h0, nr, kk in GROUPS if kk == "S"]
DUMPS = [(h0, nr) for h0, nr, kk in GROUPS if kk == "D"]
OUT2_OFF = np.cumsum([0] + [128 * nr * 144 for h0, nr in SCATTERS]).tolist()
QB = 32 * 24 * 80  # one quarter block: all 24 slots
QTOT = 4 * QB  # per dump group
STORE_LAG = 6

_cache = {}


def make_idxs():
    """idx[a, hl*464 + off_ci + col] = (hl*3+ci)*48 + (col-a) if valid else -1."""
    idx = np.full((128, 8 * ROWW), -1, dtype=np.int16)
    a = np.arange(128)
    for hl in range(8):
        for ci, off, cw in ((0, 0, 176), (1, 176, 176), (2, 352, 112)):
            s = hl * 3 + ci
            for k in range(D):
                col = a + k
                valid = col < cw
                if ci == 2:
                    valid = valid & (a < 64)
                idx[a[valid], hl * ROWW + off + col[valid]] = s * D + k
    return idx


def _emit_store(nc, out2, quads, item):
    kind, gi, tile_ = item
    if kind == "band":
        si = [j for j, (h0, nr, kk) in enumerate(GROUPS) if kk == "S"].index(gi)
        nrows = GROUPS[gi][1]
        dst = AP(out2.tensor, out2.offset + OUT2_OFF[si],
                 [[nrows * 144, 128], [1, nrows * 144]])
        nc.sync.dma_start(out=dst, in_=tile_[:, : nrows * 144])
        return
    rp = tile_.ap[0][0]
    di = [j for j, (h0, nr, kk) in enumerate(GROUPS) if kk == "D"].index(gi)
    qbase = quads.offset + di * QTOT
    for q in range(4):  # quarter q: rows [32q,32q+32), cols [32q,32q+80) of all 24 slots
        src = AP(tile_.tensor, tile_.offset + 32 * q * rp + 32 * q,
                 [[rp, 32], [176, 24], [1, 80]])
        dst = AP(quads.tensor, qbase + q * QB, [[24 * 80, 32], [80, 24], [1, 80]])
        nc.sync.dma_start(out=dst, in_=src)


def _build():
    nc = bacc.Bacc("TRN2", target_bir_lowering=False, debug=False, num_devices=8)
    left = nc.dram_tensor("left", [C, HW], mybir.dt.bfloat16, kind="ExternalInput").ap()
    right = nc.dram_tensor("right", [C, HW], mybir.dt.bfloat16, kind="ExternalInput").ap()
    idxs_in = nc.dram_tensor("idxs", [128, 8 * ROWW], mybir.dt.int16, kind="ExternalInput").ap()
    out2 = nc.dram_tensor("out2", [OUT2_OFF[-1]], mybir.dt.bfloat16,
                          kind="ExternalOutput").ap()
    quads = nc.dram_tensor("quads", [len(DUMPS) * QTOT], mybir.dt.bfloat16,
                           kind="ExternalOutput").ap()

    with tile.TileContext(nc) as tc:
        with (
            tc.tile_pool(name="io", bufs=8) as io_pool,
            tc.tile_pool(name="rect", bufs=8) as rect_pool,
            tc.tile_pool(name="band", bufs=8) as band_pool,
            tc.tile_pool(name="const", bufs=1) as const_pool,
            tc.tile_pool(name="ps", bufs=4, space="PSUM") as ps_pool,
        ):
            idx_t = const_pool.tile([128, 8 * ROWW], mybir.dt.int16)
            pending = []

            for gi, (h0, nrows, kind) in enumerate(GROUPS):
                l_t = io_pool.tile([C, 8 * W], mybir.dt.bfloat16, tag="lt")
                r_t = io_pool.tile([C, RW], mybir.dt.bfloat16, tag="rt")
                nc.sync.dma_start(out=l_t[:, : nrows * W],
                                  in_=left[:, h0 * W : (h0 + nrows) * W])
                nc.sync.dma_start(out=r_t[:, 47 : 47 + nrows * W],
                                  in_=right[:, h0 * W : (h0 + nrows) * W])
                # idx table loaded in prefix pieces so early small groups
                # can scatter before the whole table arrives
                if gi == 0:
                    nc.sync.dma_start(out=idx_t[:, : 2 * ROWW],
                                      in_=idxs_in[:, : 2 * ROWW])
                elif gi == 1:
                    nc.sync.dma_start(out=idx_t[:, 2 * ROWW : 4 * ROWW],
                                      in_=idxs_in[:, 2 * ROWW : 4 * ROWW])
                elif gi == 2:
                    nc.sync.dma_start(out=idx_t[:, 4 * ROWW :],
                                      in_=idxs_in[:, 4 * ROWW :])

                rect_g = rect_pool.tile([128, RECW], mybir.dt.bfloat16, tag="rect")
                rp = rect_g.ap[0][0]
                roww = ROWW if kind == "S" else DROWW
                for hl in range(nrows):
                    # 2 PSUM banks; chunk slots at {0,256,512}: no bank crossing.
                    g_ps = ps_pool.tile([128, 1024], mybir.dt.float32, tag="gps")
                    pp = g_ps.ap[0][0]
                    for ci, (X0, M, NMM) in enumerate(CHUNKS):
                        nc.tensor.matmul(
                            g_ps[:M, ci * 256 : ci * 256 + NMM],
                            l_t[:, hl * W + X0 : hl * W + X0 + M],
                            r_t[:, hl * W + X0 : hl * W + X0 + NMM],
                            start=True, stop=True,
                        )
                    dst_ab = AP(rect_g.tensor, rect_g.offset + hl * roww,
                                [[rp, 128], [176, 2], [1, 176]])
                    src_ab = AP(g_ps.tensor, g_ps.offset, [[pp, 128], [256, 2], [1, 176]])
                    dst_c = rect_g[:, hl * roww + 352 : hl * roww + 352 + 112]
                    src_c = g_ps[:, 512 : 512 + 112]
                    if hl % 2 == 0:
                        nc.vector.tensor_copy(dst_ab, src_ab)
                        nc.scalar.copy(dst_c, src_c)
                    else:
                        nc.scalar.copy(dst_ab, src_ab)
                        nc.vector.tensor_copy(dst_c, src_c)

                if kind == "S":
                    band_g = band_pool.tile([128, 8 * 144], mybir.dt.bfloat16, tag="band")
                    nc.gpsimd.local_scatter(
                        band_g[:, : nrows * 144], rect_g[:, : nrows * ROWW],
                        idx_t[:, : nrows * ROWW],
                        channels=128, num_elems=nrows * 144, num_idxs=nrows * ROWW,
                    )
                    pending.append(("band", gi, band_g))
                else:
                    pending.append(("dump", gi, rect_g))
                while pending and pending[0][1] <= gi - STORE_LAG:
                    _emit_store(nc, out2, quads, pending.pop(0))
            while pending:
                _emit_store(nc, out2, quads, pending.pop(0))
    nc.compile()
    return nc


def _get_nc(_mode=None):
    if "nc" not in _cache:
        _cache["nc"] = _build()
    return _cache["nc"]


def kernel(left_feature, right_feature):
    left_feature = np.asarray(left_feature, dtype=np.float32)
    right_feature = np.asarray(right_feature, dtype=np.float32)
    b, c, h, w = left_feature.shape
    assert (b, c, h, w) == (B, C, H, W)
    nc = _get_nc()
    idx = make_idxs()
    in_maps = []
    for i in range(B):
        lf = (left_feature[i].reshape(C, HW) * np.float32(1.0 / C)).astype(ml_dtypes.bfloat16)
        rf = right_feature[i].reshape(C, HW).astype(ml_dtypes.bfloat16)
        in_maps.append({
            "left": np.ascontiguousarray(lf),
            "right": np.ascontiguousarray(rf),
            "idxs": idx,
        })
    trace = bool(os.environ.get("KERNEL_TRACE"))
    res = run_bass_kernel_spmd(nc, in_maps, core_ids=list(range(B)), trace=trace)
    if trace:
        print("HW exec time:", res.exec_time_ns, "ns")
    outs = []
    a32 = np.arange(32)
    for i in range(B):
        vol = np.empty((D, H, W), dtype=np.float32)
        raw = np.asarray(res.results[i]["out2"]).astype(np.float32)
        for si, (h0, nr) in enumerate(SCATTERS):
            band = raw[OUT2_OFF[si] : OUT2_OFF[si + 1]].reshape(128, nr, 3, D)
            v = band.transpose(3, 1, 2, 0)  # [k, hl, ci, a]
            vol[:, h0 : h0 + nr, :] = v.reshape(D, nr, 3 * 128)[:, :, :W]
        qall = np.asarray(res.results[i]["quads"]).astype(np.float32).reshape(
            len(DUMPS), 4, 32, 24, 80)  # [dg, quarter, a32, slot=hl*3+ci, col80]
        for di, (h0, nr) in enumerate(DUMPS):
            for k in range(D):
                sel = (a32 + k)[None, :, None, None]
                blk = np.take_along_axis(qall[di], sel, axis=3)[:, :, :, 0]  # [4, 32, 24]
                bnd = blk.reshape(128, 8, 3)  # [a, hl, ci]
                vv = bnd.transpose(1, 2, 0).reshape(8, 3 * 128)[:, :W]  # [hl, x]
                vol[k, h0 : h0 + nr, :] = vv
        outs.append(vol[::-1])  # k = 47 - i
    out = np.stack(outs, axis=0)
    for i in range(1, D):
        out[:, i, :, :i] = 0.0
    return out


if __name__ == "__main__":
    rng = np.random.default_rng(0)
    lf = rng.standard_normal((B, C, H, W), dtype=np.float32)
    rf = rng.standard_normal((B, C, H, W), dtype=np.float32)
    got = kernel(lf, rf)
    for (bb, i, hh, xx) in [(0, 0, 0, 0), (0, 5, 10, 100), (1, 47, 95, 319), (2, 47, 3, 10),
                            (3, 20, 85, 200), (7, 1, 90, 300), (5, 30, 35, 128)]:
        want = float(np.dot(lf[bb, :, hh, xx], rf[bb, :, hh, xx - i]) / C) if xx >= i else 0.0
        print((bb, i, hh, xx), "got", got[bb, i, hh, xx], "want", want)
